# revision 11
# baseline (speedup 1.0000x reference)
"""Trainium2 Bass kernel for nn_ConvBlock (SepGconv + LayerNorm + GELU MLP).

Computes, for full inputs:
    a   = einsum('bsc,brsd,dc->brc', x, kernel_basis, kernel_W) + conv_bias
    a   = LayerNorm(a) * ln_scale + ln_bias          (over channels, eps=1e-6)
    out = gelu_tanh(a @ W1 + b1) @ W2 + b2

Shapes: B=2, N=1024 (R=S=N), H=64, D=32, WF=4.

Sharding: the (B*R)=2048 output rows are split into 8 contiguous shards of
256 rows, one per NeuronCore. Each core reads its 32 MB kernel_basis shard
once (memory-bound), contracts over all S on-chip, and runs the small
LN/MLP tail locally. x / weights are replicated.

Per-core dataflow:
  - kernel_basis shard is host-permuted into supertiles of 8 rows laid out
    (128 s-partitions, 8 s-chunks, 8 rows, 32 d) so each supertile is one
    fully-linear 1 MB DMA.
  - For each 4-row group: 8 accumulating fp32 matmuls
        psum[(r^,d), c] += kb[s,(r^,d)]^T @ x[s, c]     (K=128, M=128, N=64)
  - DVE evacuates psum while multiplying by W broadcast (128,64):
        mw[(r^,d), c] = psum * W[d, c]
  - A shifted 0/1 selection matmul reduces over d and scatters the 4 rows
    into their block positions, accumulating 32 groups into one
    (128 rows, 64 ch) psum block:
        a[4*g0+r^, c] += sum_d mw[(r^,d), c]
  - conv_bias add (fused with psum evac), LayerNorm via bn_stats/bn_aggr,
    PE transpose, W1/W2 matmuls with Gelu_apprx_tanh between.
"""

import os

import numpy as np

import concourse.bass as bass
import concourse.tile as tile
from concourse import mybir
from concourse.bass_utils import run_bass_kernel_spmd


def _ensure_axon_hooks():
    """bass_utils imports antenv.axon_hooks when trace=True under axon; some
    images ship antenv without that module. Register a functional stand-in
    (driving NTFF capture via libaxon_pjrt.so) so tracing works, degrading
    to hook=None (no trace, run still works) if the .so is unavailable."""
    try:
        import antenv.axon_hooks  # noqa: F401

        return
    except ImportError:
        pass
    import sys
    import types

    import antenv

    mod = types.ModuleType("antenv.axon_hooks")
    mod._hook = None

    def set_axon_ntff_profile_hook(h):
        mod._hook = h

    def get_axon_ntff_profile_hook():
        if mod._hook is None:
            try:
                from trn_agent_boot.trn_boot import _ntff_profile_via_ctypes

                so_path = "/opt/axon/libaxon_pjrt.so"
                if os.path.exists(so_path):
                    mod._hook = _ntff_profile_via_ctypes(so_path)
            except Exception:
                mod._hook = None
        return mod._hook

    mod.set_axon_ntff_profile_hook = set_axon_ntff_profile_hook
    mod.get_axon_ntff_profile_hook = get_axon_ntff_profile_hook
    sys.modules["antenv.axon_hooks"] = mod
    antenv.axon_hooks = mod


_ensure_axon_hooks()

F32 = mybir.dt.float32

B, N, H, D, WF = 2, 1024, 64, 32, 4
NCORES = 8
ROWS_PER_CORE = (B * N) // NCORES  # 256
N_SUPER = ROWS_PER_CORE // 8  # 32 supertiles of 8 rows
N_KCHUNK = N // 128  # 8 s-chunks of 128
FH = WF * H  # 256
LN_EPS = 1e-6

# Selection matrix width: slices [124-4*g0 : 252-4*g0] for g0 in [0,32)
SFULL_W = 252

_NC_CACHE = None
LAST_EXEC_NS = None


def _build_nc(split_waits=True):
    nc = bass.Bass(target_bir_lowering=False)

    kbp = nc.dram_tensor("kbp", [N_SUPER, 128, 8, 8, D], F32, kind="ExternalInput")
    xp = nc.dram_tensor("xp", [128, N_KCHUNK, H], F32, kind="ExternalInput")
    w_bcast = nc.dram_tensor("w_bcast", [128, H], F32, kind="ExternalInput")
    sfull = nc.dram_tensor("sfull", [128, SFULL_W], F32, kind="ExternalInput")
    ident = nc.dram_tensor("ident", [128, 128], F32, kind="ExternalInput")
    cb_bcast = nc.dram_tensor("cb_bcast", [128, H], F32, kind="ExternalInput")
    lns_bcast = nc.dram_tensor("lns_bcast", [128, H], F32, kind="ExternalInput")
    lnb_bcast = nc.dram_tensor("lnb_bcast", [128, H], F32, kind="ExternalInput")
    w1 = nc.dram_tensor("w1", [H, FH], F32, kind="ExternalInput")
    b1p = nc.dram_tensor("b1p", [128, 2], F32, kind="ExternalInput")
    w2p = nc.dram_tensor("w2p", [128, 2, H], F32, kind="ExternalInput")
    b2_bcast = nc.dram_tensor("b2_bcast", [128, H], F32, kind="ExternalInput")
    out = nc.dram_tensor("out", [ROWS_PER_CORE, H], F32, kind="ExternalOutput")

    with tile.TileContext(nc) as tc:
        with (
            tc.tile_pool(name="consts", bufs=1) as consts,
            tc.tile_pool(name="kb", bufs=6) as kb_pool,
            tc.tile_pool(name="mw", bufs=3) as mw_pool,
            tc.tile_pool(name="work", bufs=2) as work,
            tc.tile_pool(name="pm", bufs=2, space="PSUM") as pm_pool,
            tc.tile_pool(name="pblk", bufs=2, space="PSUM") as pblk_pool,
            tc.tile_pool(name="pmlp", bufs=3, space="PSUM") as pmlp_pool,
        ):
            # ---- load constants ----
            x_sb = consts.tile([128, N_KCHUNK, H], F32)
            nc.sync.dma_start(out=x_sb, in_=xp[:, :, :])
            w_sb = consts.tile([128, H], F32)
            nc.sync.dma_start(out=w_sb, in_=w_bcast[:, :])
            s_sb = consts.tile([128, SFULL_W], F32)
            nc.sync.dma_start(out=s_sb, in_=sfull[:, :])
            id_sb = consts.tile([128, 128], F32)
            nc.sync.dma_start(out=id_sb, in_=ident[:, :])
            cb_sb = consts.tile([128, H], F32)
            nc.sync.dma_start(out=cb_sb, in_=cb_bcast[:, :])
            lns_sb = consts.tile([128, H], F32)
            nc.sync.dma_start(out=lns_sb, in_=lns_bcast[:, :])
            lnb_sb = consts.tile([128, H], F32)
            nc.sync.dma_start(out=lnb_sb, in_=lnb_bcast[:, :])
            w1_sb = consts.tile([H, FH], F32)
            nc.sync.dma_start(out=w1_sb, in_=w1[:, :])
            b1_sb = consts.tile([128, 2], F32)
            nc.sync.dma_start(out=b1_sb, in_=b1p[:, :])
            w2_sb = consts.tile([128, 2, H], F32)
            nc.sync.dma_start(out=w2_sb, in_=w2p[:, :, :])
            b2_sb = consts.tile([128, H], F32)
            nc.sync.dma_start(out=b2_sb, in_=b2_bcast[:, :])
            eps_sb = consts.tile([128, 1], F32)
            nc.vector.memset(eps_sb, LN_EPS)

            for blk in range(2):
                a_psum = pblk_pool.tile([128, H], F32)
                for g_local in range(N_SUPER // 2):
                    g = blk * (N_SUPER // 2) + g_local
                    kb_t = kb_pool.tile([128, N_KCHUNK, 8, D], F32)
                    nc.sync.dma_start(out=kb_t, in_=kbp[g, :, :, :, :])
                    for rg in range(2):
                        g0 = 2 * g_local + rg  # group index within block
                        pm = pm_pool.tile([128, H], F32)
                        for k in range(N_KCHUNK):
                            nc.tensor.matmul(
                                pm,
                                lhsT=kb_t[:, k, 4 * rg : 4 * rg + 4, :],
                                rhs=x_sb[:, k, :],
                                start=(k == 0),
                                stop=(k == N_KCHUNK - 1),
                            )
                        mw = mw_pool.tile([128, H], F32)
                        nc.vector.tensor_mul(mw, pm, w_sb)
                        nc.tensor.matmul(
                            a_psum,
                            lhsT=s_sb[:, 124 - 4 * g0 : 252 - 4 * g0],
                            rhs=mw,
                            start=(g0 == 0),
                            stop=(g0 == 31),
                        )

                # ---- evac + conv_bias ----
                a_sb = work.tile([128, H], F32)
                nc.vector.tensor_add(a_sb, a_psum, cb_sb)

                # ---- LayerNorm over channels ----
                stats = work.tile([128, 6], F32)
                nc.vector.bn_stats(out=stats, in_=a_sb)
                mv = work.tile([128, 2], F32)
                nc.vector.bn_aggr(out=mv, in_=stats)
                std = work.tile([128, 1], F32)
                nc.scalar.activation(
                    out=std,
                    in_=mv[:, 1:2],
                    func=mybir.ActivationFunctionType.Sqrt,
                    bias=eps_sb,
                    scale=1.0,
                )
                rstd = work.tile([128, 1], F32)
                nc.vector.reciprocal(out=rstd, in_=std)
                a_n = work.tile([128, H], F32)
                nc.vector.tensor_scalar(
                    out=a_n,
                    in0=a_sb,
                    scalar1=mv[:, 0:1],
                    scalar2=rstd,
                    op0=mybir.AluOpType.subtract,
                    op1=mybir.AluOpType.mult,
                )
                a_g = work.tile([128, H], F32)
                nc.vector.tensor_mul(a_g, a_n, lns_sb)
                a_ln = work.tile([128, H], F32)
                nc.vector.tensor_add(a_ln, a_g, lnb_sb)

                # ---- transpose a_ln -> (H, 128) ----
                pT = pmlp_pool.tile([H, 128], F32, bufs=1)
                nc.tensor.transpose(pT, a_ln, id_sb)
                alnT = work.tile([H, 128], F32)
                nc.scalar.copy(out=alnT, in_=pT)

                # ---- MLP: hT = gelu(W1^T @ alnT + b1) ----
                hT = work.tile([128, 2, 128], F32)
                for half in range(2):
                    ph = pmlp_pool.tile([128, 128], F32, bufs=2)
                    nc.tensor.matmul(
                        ph,
                        lhsT=w1_sb[:, 128 * half : 128 * (half + 1)],
                        rhs=alnT,
                        start=True,
                        stop=True,
                    )
                    nc.scalar.activation(
                        out=hT[:, half, :],
                        in_=ph,
                        func=mybir.ActivationFunctionType.Gelu_apprx_tanh,
                        bias=b1_sb[:, half : half + 1],
                        scale=1.0,
                    )

                # ---- out = hT^T @ W2 + b2 ----
                po = pmlp_pool.tile([128, H], F32, bufs=1)
                for half in range(2):
                    nc.tensor.matmul(
                        po,
                        lhsT=hT[:, half, :],
                        rhs=w2_sb[:, half, :],
                        start=(half == 0),
                        stop=(half == 1),
                    )
                o_sb = work.tile([128, H], F32)
                nc.vector.tensor_add(o_sb, po, b2_sb)
                nc.sync.dma_start(out=out[128 * blk : 128 * (blk + 1), :], in_=o_sb)

    if split_waits:
        _split_matmul_waits(nc)
    return nc


def _split_matmul_waits(nc):
    """This walrus build rejects engine instructions carrying more than one
    semaphore wait ("Too many sync wait commands"). Peel all but the last
    wait off onto same-engine NoOps inserted immediately before the
    instruction — NoOps execute in queue order on the same sequencer, so the
    wait semantics are unchanged."""
    f = nc.m.functions[0]
    nop_id = 0
    for blk in f.blocks:
        insts = list(blk.instructions)
        out = []
        changed = False
        for inst in insts:
            si = inst.sync_info
            if (
                si is not None
                and si.on_wait is not None
                and len(si.on_wait) > 1
                and getattr(inst, "engine", None) is not None
            ):
                waits = list(si.on_wait)
                for w in waits[:-1]:
                    nop = mybir.InstNoOp(
                        name=f"I-mmwait-{nop_id}",
                        engine=inst.engine,
                        ins=[],
                        outs=[],
                        sync_info=mybir.SyncInfo(on_wait=[w], on_update=[]),
                    )
                    nop_id += 1
                    out.append(nop)
                inst.sync_info = mybir.SyncInfo(
                    on_wait=[waits[-1]], on_update=list(si.on_update or [])
                )
                changed = True
            out.append(inst)
        if changed:
            blk.instructions = out


def _get_nc():
    global _NC_CACHE
    if _NC_CACHE is None:
        _NC_CACHE = _build_nc()
    return _NC_CACHE


def kernel(
    x,
    kernel_basis,
    kernel_W,
    conv_bias,
    ln_scale,
    ln_bias,
    W1,
    b1,
    W2,
    b2,
):
    global LAST_EXEC_NS
    x = np.ascontiguousarray(np.asarray(x, np.float32))
    kb = np.ascontiguousarray(np.asarray(kernel_basis, np.float32))
    kernel_W = np.asarray(kernel_W, np.float32)
    conv_bias = np.asarray(conv_bias, np.float32)
    ln_scale = np.asarray(ln_scale, np.float32)
    ln_bias = np.asarray(ln_bias, np.float32)
    W1 = np.asarray(W1, np.float32)
    b1 = np.asarray(b1, np.float32)
    W2 = np.asarray(W2, np.float32)
    b2 = np.asarray(b2, np.float32)

    # Shared (replicated) small tensors, pre-laid-out for the device.
    w_bcast = np.ascontiguousarray(np.tile(kernel_W, (4, 1)))  # (128, H)
    sfull = np.zeros((128, SFULL_W), np.float32)
    sfull[np.arange(128), 124 + np.arange(128) // 32] = 1.0
    ident = np.eye(128, dtype=np.float32)
    cb_bcast = np.ascontiguousarray(np.broadcast_to(conv_bias, (128, H)))
    lns_bcast = np.ascontiguousarray(np.broadcast_to(ln_scale, (128, H)))
    lnb_bcast = np.ascontiguousarray(np.broadcast_to(ln_bias, (128, H)))
    b1p = np.ascontiguousarray(b1.reshape(2, 128).T)  # (128, 2)
    w2p = np.ascontiguousarray(W2.reshape(2, 128, H).transpose(1, 0, 2))
    b2_bcast = np.ascontiguousarray(np.broadcast_to(b2, (128, H)))
    # x[b] as (s-part, s-chunk, c)
    xps = [
        np.ascontiguousarray(x[b].reshape(N_KCHUNK, 128, H).transpose(1, 0, 2))
        for b in range(B)
    ]

    kbf = kb.reshape(B * N, N, D)
    in_maps = []
    for c in range(NCORES):
        shard = kbf[c * ROWS_PER_CORE : (c + 1) * ROWS_PER_CORE]
        # (g, r8, k, p, d) -> (g, p, k, r8, d)
        kbp = np.ascontiguousarray(
            shard.reshape(N_SUPER, 8, N_KCHUNK, 128, D).transpose(0, 3, 2, 1, 4)
        )
        in_maps.append(
            dict(
                kbp=kbp,
                xp=xps[c // (NCORES // B)],
                w_bcast=w_bcast,
                sfull=sfull,
                ident=ident,
                cb_bcast=cb_bcast,
                lns_bcast=lns_bcast,
                lnb_bcast=lnb_bcast,
                w1=W1,
                b1p=b1p,
                w2p=w2p,
                b2_bcast=b2_bcast,
            )
        )

    nc = _get_nc()
    trace = bool(os.environ.get("KERNEL_BASS_TRACE"))
    res = run_bass_kernel_spmd(nc, in_maps, core_ids=list(range(NCORES)), trace=trace)
    LAST_EXEC_NS = res.exec_time_ns

    outs = np.concatenate([res.results[c]["out"] for c in range(NCORES)], axis=0)
    return outs.reshape(B, N, H)


# revision 12
# speedup vs baseline: 2.3464x; 2.3464x over previous
"""Trainium2 Bass kernel for nn_ConvBlock (SepGconv + LayerNorm + GELU MLP).

Computes, for full inputs:
    a   = einsum('bsc,brsd,dc->brc', x, kernel_basis, kernel_W) + conv_bias
    a   = LayerNorm(a) * ln_scale + ln_bias          (over channels, eps=1e-6)
    out = gelu_tanh(a @ W1 + b1) @ W2 + b2

Shapes: B=2, N=1024 (R=S=N), H=64, D=32, WF=4.

Sharding: the (B*R)=2048 output rows split into 8 contiguous shards of 256
rows, one per NeuronCore. Each core reads its kernel_basis shard once
(memory-bound), contracts over all S on-chip, and runs the LN/MLP tail
locally. x / weights are replicated.

Precision/perf strategy: the PE's weight-load port is slow for fp32
(measured ~427 ns per K=128 reload), so the 32 MB/core kernel_basis shard
must stream through the fast moving-operand port in bf16. To keep ~fp32
accuracy both operands are split hi/lo in bf16:
    kb = kbh + kbl,  x = xh + xl,
    a ~= xh.kbh + xl.kbh + xh.kbl   (xl.kbl ~ 2^-18 is dropped)
Each matmul is  psum[c, (r,d)] += x[s,c]^T @ kb[s,(r,d)]  with N=512
(16 rows x 32 d), K=128 s-chunk, M=64 channels; x tiles are the (tiny)
stationary weights. The d-reduction with kernel_W happens on DVE:
multiply by W broadcast, then a free-axis tensor_reduce over d, yielding
aT (64 ch, 256 rows). LayerNorm runs in this transposed space (stats via
two ones-matmuls, partition-broadcast via a K=1 matmul), and the MLP
consumes aT directly (h = W1^T @ aT), so no transposes are needed.
"""

import os

import numpy as np

import concourse.bass as bass
import concourse.tile as tile
from concourse import mybir
from concourse.bass_utils import run_bass_kernel_spmd


def _ensure_axon_hooks():
    """bass_utils imports antenv.axon_hooks when trace=True under axon; some
    images ship antenv without that module. Register a functional stand-in
    (driving NTFF capture via libaxon_pjrt.so) so tracing works, degrading
    to hook=None (no trace, run still works) if the .so is unavailable."""
    try:
        import antenv.axon_hooks  # noqa: F401

        return
    except ImportError:
        pass
    import sys
    import types

    import antenv

    mod = types.ModuleType("antenv.axon_hooks")
    mod._hook = None

    def set_axon_ntff_profile_hook(h):
        mod._hook = h

    def get_axon_ntff_profile_hook():
        if mod._hook is None:
            try:
                from trn_agent_boot.trn_boot import _ntff_profile_via_ctypes

                so_path = "/opt/axon/libaxon_pjrt.so"
                if os.path.exists(so_path):
                    mod._hook = _ntff_profile_via_ctypes(so_path)
            except Exception:
                mod._hook = None
        return mod._hook

    mod.set_axon_ntff_profile_hook = set_axon_ntff_profile_hook
    mod.get_axon_ntff_profile_hook = get_axon_ntff_profile_hook
    sys.modules["antenv.axon_hooks"] = mod
    antenv.axon_hooks = mod


_ensure_axon_hooks()

F32 = mybir.dt.float32
BF16 = mybir.dt.bfloat16

B, N, H, D, WF = 2, 1024, 64, 32, 4
NCORES = 8
ROWS_PER_CORE = (B * N) // NCORES  # 256
RB = 16  # rows per j-block
N_JBLK = ROWS_PER_CORE // RB  # 16
N_KCHUNK = N // 128  # 8 s-chunks of 128
FH = WF * H  # 256
LN_EPS = 1e-6

_NC_CACHE = None
LAST_EXEC_NS = None


def _build_nc(split_waits=True):
    nc = bass.Bass(target_bir_lowering=False)

    kbh = nc.dram_tensor("kbh", [N_JBLK, 128, N_KCHUNK, RB, D], BF16, kind="ExternalInput")
    kbl = nc.dram_tensor("kbl", [N_JBLK, 128, N_KCHUNK, RB, D], BF16, kind="ExternalInput")
    xhp = nc.dram_tensor("xhp", [128, N_KCHUNK, H], BF16, kind="ExternalInput")
    xlp = nc.dram_tensor("xlp", [128, N_KCHUNK, H], BF16, kind="ExternalInput")
    wb2 = nc.dram_tensor("wb2", [H, RB * D], F32, kind="ExternalInput")
    cbT = nc.dram_tensor("cbT", [H, 1], F32, kind="ExternalInput")
    lnsT = nc.dram_tensor("lnsT", [H, 1], F32, kind="ExternalInput")
    lnbT = nc.dram_tensor("lnbT", [H, 1], F32, kind="ExternalInput")
    w1 = nc.dram_tensor("w1", [H, FH], F32, kind="ExternalInput")
    b1p = nc.dram_tensor("b1p", [128, 2], F32, kind="ExternalInput")
    w2p = nc.dram_tensor("w2p", [128, 2, H], F32, kind="ExternalInput")
    b2_bcast = nc.dram_tensor("b2_bcast", [128, H], F32, kind="ExternalInput")
    out = nc.dram_tensor("out", [ROWS_PER_CORE, H], F32, kind="ExternalOutput")

    with tile.TileContext(nc) as tc:
        with (
            tc.tile_pool(name="consts", bufs=1) as consts,
            tc.tile_pool(name="kbhp", bufs=3) as kbh_pool,
            tc.tile_pool(name="kblp", bufs=3) as kbl_pool,
            tc.tile_pool(name="mwp", bufs=2) as mw_pool,
            tc.tile_pool(name="work", bufs=2) as work,
            tc.tile_pool(name="pmain", bufs=2, space="PSUM") as pmain,
            tc.tile_pool(name="ptail", bufs=1, space="PSUM") as ptail,
        ):
            # ---- constants ----
            xh_sb = consts.tile([128, N_KCHUNK, H], BF16)
            nc.sync.dma_start(out=xh_sb, in_=xhp[:, :, :])
            xl_sb = consts.tile([128, N_KCHUNK, H], BF16)
            nc.sync.dma_start(out=xl_sb, in_=xlp[:, :, :])
            wb_sb = consts.tile([H, RB * D], F32)
            nc.sync.dma_start(out=wb_sb, in_=wb2[:, :])
            cb_sb = consts.tile([H, 1], F32)
            nc.sync.dma_start(out=cb_sb, in_=cbT[:, :])
            lns_sb = consts.tile([H, 1], F32)
            nc.sync.dma_start(out=lns_sb, in_=lnsT[:, :])
            lnb_sb = consts.tile([H, 1], F32)
            nc.sync.dma_start(out=lnb_sb, in_=lnbT[:, :])
            w1_sb = consts.tile([H, FH], F32)
            nc.sync.dma_start(out=w1_sb, in_=w1[:, :])
            b1_sb = consts.tile([128, 2], F32)
            nc.sync.dma_start(out=b1_sb, in_=b1p[:, :])
            w2_sb = consts.tile([128, 2, H], F32)
            nc.sync.dma_start(out=w2_sb, in_=w2p[:, :, :])
            b2_sb = consts.tile([128, H], F32)
            nc.sync.dma_start(out=b2_sb, in_=b2_bcast[:, :])
            ones64 = consts.tile([H, 1], F32)
            nc.vector.memset(ones64, 1.0)
            ones1 = consts.tile([1, H], F32)
            nc.vector.memset(ones1, 1.0)
            eps1 = consts.tile([1, 1], F32)
            nc.vector.memset(eps1, LN_EPS)
            aT = consts.tile([H, ROWS_PER_CORE], F32)

            # ---- main contraction ----
            for j in range(N_JBLK):
                kbh_t = kbh_pool.tile([128, N_KCHUNK, RB, D], BF16)
                nc.sync.dma_start(out=kbh_t, in_=kbh[j, :, :, :, :])
                kbl_t = kbl_pool.tile([128, N_KCHUNK, RB, D], BF16)
                nc.sync.dma_start(out=kbl_t, in_=kbl[j, :, :, :, :])
                ps = pmain.tile([H, RB * D], F32)
                for k in range(N_KCHUNK):
                    nc.tensor.matmul(
                        ps,
                        lhsT=xh_sb[:, k, :],
                        rhs=kbh_t[:, k, :, :],
                        start=(k == 0),
                        stop=False,
                    )
                    nc.tensor.matmul(
                        ps, lhsT=xh_sb[:, k, :], rhs=kbl_t[:, k, :, :],
                        start=False, stop=False,
                    )
                    nc.tensor.matmul(
                        ps, lhsT=xl_sb[:, k, :], rhs=kbh_t[:, k, :, :],
                        start=False, stop=(k == N_KCHUNK - 1),
                    )
                mw = mw_pool.tile([H, RB, D], F32)
                nc.vector.tensor_mul(
                    mw.rearrange("p a b -> p (a b)"), ps, wb_sb
                )
                nc.vector.tensor_reduce(
                    out=aT[:, RB * j : RB * (j + 1)],
                    in_=mw,
                    axis=mybir.AxisListType.X,
                    op=mybir.AluOpType.add,
                )

            # ---- conv_bias + LayerNorm in transposed (ch-partition) space ----
            aTb = work.tile([H, ROWS_PER_CORE], F32)
            nc.vector.tensor_scalar(
                out=aTb, in0=aT, scalar1=cb_sb, scalar2=None,
                op0=mybir.AluOpType.add,
            )
            sq = work.tile([H, ROWS_PER_CORE], F32)
            nc.vector.tensor_mul(sq, aTb, aTb)
            ps_s1 = ptail.tile([1, ROWS_PER_CORE], F32, bufs=1)
            nc.tensor.matmul(ps_s1, lhsT=ones64, rhs=aTb, start=True, stop=True)
            ps_s2 = ptail.tile([1, ROWS_PER_CORE], F32, bufs=1)
            nc.tensor.matmul(ps_s2, lhsT=ones64, rhs=sq, start=True, stop=True)
            # rp = [rstd | mean*rstd] staged in one (1, 512) tile
            rp = work.tile([1, 2 * ROWS_PER_CORE], F32)
            mean = work.tile([1, ROWS_PER_CORE], F32)
            nc.vector.tensor_scalar(
                out=mean, in0=ps_s1, scalar1=1.0 / H, scalar2=None,
                op0=mybir.AluOpType.mult,
            )
            msq = work.tile([1, ROWS_PER_CORE], F32)
            nc.vector.tensor_scalar(
                out=msq, in0=ps_s2, scalar1=1.0 / H, scalar2=None,
                op0=mybir.AluOpType.mult,
            )
            m2 = work.tile([1, ROWS_PER_CORE], F32)
            nc.vector.tensor_mul(m2, mean, mean)
            var = work.tile([1, ROWS_PER_CORE], F32)
            nc.vector.tensor_sub(var, msq, m2)
            std = work.tile([1, ROWS_PER_CORE], F32)
            nc.scalar.activation(
                out=std, in_=var,
                func=mybir.ActivationFunctionType.Sqrt,
                bias=eps1, scale=1.0,
            )
            nc.vector.reciprocal(out=rp[:, 0:ROWS_PER_CORE], in_=std)
            nc.vector.tensor_mul(
                rp[:, ROWS_PER_CORE : 2 * ROWS_PER_CORE],
                mean,
                rp[:, 0:ROWS_PER_CORE],
            )
            ps_bc = ptail.tile([H, 2 * ROWS_PER_CORE], F32, bufs=1)
            nc.tensor.matmul(ps_bc, lhsT=ones1, rhs=rp, start=True, stop=True)
            t1 = work.tile([H, ROWS_PER_CORE], F32)
            nc.vector.tensor_mul(t1, aTb, ps_bc[:, 0:ROWS_PER_CORE])
            t2 = work.tile([H, ROWS_PER_CORE], F32)
            nc.vector.tensor_sub(t2, t1, ps_bc[:, ROWS_PER_CORE : 2 * ROWS_PER_CORE])
            aln = work.tile([H, ROWS_PER_CORE], F32)
            nc.vector.tensor_scalar(
                out=aln, in0=t2, scalar1=lns_sb, scalar2=lnb_sb,
                op0=mybir.AluOpType.mult, op1=mybir.AluOpType.add,
            )

            # ---- MLP: hT = gelu(W1^T @ aln + b1);  out = hT^T @ W2 + b2 ----
            hT = work.tile([128, 2, ROWS_PER_CORE], F32)
            for half in range(2):
                ph = ptail.tile([128, ROWS_PER_CORE], F32, bufs=2)
                nc.tensor.matmul(
                    ph,
                    lhsT=w1_sb[:, 128 * half : 128 * (half + 1)],
                    rhs=aln,
                    start=True,
                    stop=True,
                )
                nc.scalar.activation(
                    out=hT[:, half, :],
                    in_=ph,
                    func=mybir.ActivationFunctionType.Gelu_apprx_tanh,
                    bias=b1_sb[:, half : half + 1],
                    scale=1.0,
                )
            for rb in range(2):
                po = ptail.tile([128, H], F32, bufs=1)
                for half in range(2):
                    nc.tensor.matmul(
                        po,
                        lhsT=hT[:, half, 128 * rb : 128 * (rb + 1)],
                        rhs=w2_sb[:, half, :],
                        start=(half == 0),
                        stop=(half == 1),
                    )
                o_sb = work.tile([128, H], F32)
                nc.vector.tensor_add(o_sb, po, b2_sb)
                nc.sync.dma_start(out=out[128 * rb : 128 * (rb + 1), :], in_=o_sb)

    if split_waits:
        _split_matmul_waits(nc)
    return nc


def _split_matmul_waits(nc):
    """This walrus build rejects engine instructions carrying more than one
    semaphore wait ("Too many sync wait commands"). Peel all but the last
    wait off onto same-engine NoOps inserted immediately before the
    instruction — NoOps execute in queue order on the same sequencer, so the
    wait semantics are unchanged."""
    f = nc.m.functions[0]
    nop_id = 0
    for blk in f.blocks:
        insts = list(blk.instructions)
        out = []
        changed = False
        for inst in insts:
            si = inst.sync_info
            if (
                si is not None
                and si.on_wait is not None
                and len(si.on_wait) > 1
                and getattr(inst, "engine", None) is not None
            ):
                waits = list(si.on_wait)
                for w in waits[:-1]:
                    nop = mybir.InstNoOp(
                        name=f"I-mmwait-{nop_id}",
                        engine=inst.engine,
                        ins=[],
                        outs=[],
                        sync_info=mybir.SyncInfo(on_wait=[w], on_update=[]),
                    )
                    nop_id += 1
                    out.append(nop)
                inst.sync_info = mybir.SyncInfo(
                    on_wait=[waits[-1]], on_update=list(si.on_update or [])
                )
                changed = True
            out.append(inst)
        if changed:
            blk.instructions = out


def _get_nc():
    global _NC_CACHE
    if _NC_CACHE is None:
        _NC_CACHE = _build_nc()
    return _NC_CACHE


def _prep_shared(kernel_W, conv_bias, ln_scale, ln_bias, W1, b1, W2, b2):
    import ml_dtypes  # noqa: F401

    # wb2[c, r^*D + d] = W[d, c]
    wb2 = np.ascontiguousarray(
        np.tile(kernel_W.T, (1, RB)).astype(np.float32)
    )  # (H, RB*D): W.T is (H=c, D=d); tile along free => [d-block repeated RB times]
    cbT = np.ascontiguousarray(conv_bias.reshape(H, 1))
    lnsT = np.ascontiguousarray(ln_scale.reshape(H, 1))
    lnbT = np.ascontiguousarray(ln_bias.reshape(H, 1))
    b1p = np.ascontiguousarray(b1.reshape(2, 128).T)
    w2p = np.ascontiguousarray(W2.reshape(2, 128, H).transpose(1, 0, 2))
    b2b = np.ascontiguousarray(np.broadcast_to(b2, (128, H)))
    return dict(
        wb2=wb2, cbT=cbT, lnsT=lnsT, lnbT=lnbT,
        w1=np.ascontiguousarray(W1), b1p=b1p, w2p=w2p, b2_bcast=b2b,
    )


def _split_hi_lo(a):
    import ml_dtypes

    hi = a.astype(ml_dtypes.bfloat16)
    lo = (a - hi.astype(np.float32)).astype(ml_dtypes.bfloat16)
    return hi, lo


def _prep_x(xb):
    # (N, H) -> (128, k, H) with s = 128*k + p
    xh, xl = _split_hi_lo(xb)
    f = lambda t: np.ascontiguousarray(t.reshape(N_KCHUNK, 128, H).transpose(1, 0, 2))
    return f(xh), f(xl)


def _prep_kb_shard(shard):
    # shard (256, 1024, 32) -> (j, p, k, r^, d)
    hi, lo = _split_hi_lo(shard)
    f = lambda t: np.ascontiguousarray(
        t.reshape(N_JBLK, RB, N_KCHUNK, 128, D).transpose(0, 3, 2, 1, 4)
    )
    return f(hi), f(lo)


def kernel(
    x,
    kernel_basis,
    kernel_W,
    conv_bias,
    ln_scale,
    ln_bias,
    W1,
    b1,
    W2,
    b2,
):
    global LAST_EXEC_NS
    x = np.ascontiguousarray(np.asarray(x, np.float32))
    kb = np.ascontiguousarray(np.asarray(kernel_basis, np.float32))
    shared = _prep_shared(
        np.asarray(kernel_W, np.float32),
        np.asarray(conv_bias, np.float32),
        np.asarray(ln_scale, np.float32),
        np.asarray(ln_bias, np.float32),
        np.asarray(W1, np.float32),
        np.asarray(b1, np.float32),
        np.asarray(W2, np.float32),
        np.asarray(b2, np.float32),
    )
    xps = [_prep_x(x[b]) for b in range(B)]

    kbf = kb.reshape(B * N, N, D)
    in_maps = []
    for c in range(NCORES):
        hi, lo = _prep_kb_shard(kbf[c * ROWS_PER_CORE : (c + 1) * ROWS_PER_CORE])
        xh, xl = xps[c // (NCORES // B)]
        in_maps.append(dict(kbh=hi, kbl=lo, xhp=xh, xlp=xl, **shared))

    nc = _get_nc()
    trace = bool(os.environ.get("KERNEL_BASS_TRACE"))
    res = run_bass_kernel_spmd(nc, in_maps, core_ids=list(range(NCORES)), trace=trace)
    LAST_EXEC_NS = res.exec_time_ns

    outs = np.concatenate([res.results[c]["out"] for c in range(NCORES)], axis=0)
    return outs.reshape(B, N, H)
